# revision 16
# baseline (speedup 1.0000x reference)
"""Trainium2 Bass kernel for nn_GRUGen: GRU encoder + 2-layer decoder GRU with
global mixing + output heads. 8-way batch-sharded SPMD (b=8 per core).

Layout convention: everything transposed — feature dim on partitions, batch on
free dim. Recurrent matmuls are weight-stationary (bf16, FWL) producing gates
in [gate-chunk partitions, batch] layout so elementwise runs at full partition
utilization. Decoder runs as two full scans (layer 0, then layer 1) with the
inter-layer W1x projection done as blocked parallel matmuls; the per-sample
global-mixing constants are folded on the host (W0h@W0m etc).
"""
import sys
sys.path.insert(0, "/opt/trn_rl_repo")

import numpy as np
import ml_dtypes

import concourse.bass as bass
import concourse.mybir as mybir
import concourse.tile as tile
from concourse import bacc
from concourse.bass_utils import run_bass_kernel_spmd

F32 = mybir.dt.float32
BF16 = mybir.dt.bfloat16
U8 = mybir.dt.uint8
BF = ml_dtypes.bfloat16

N_CORES = 8
B, T, D = 64, 512, 528
E, Fdim = 8, 64
HE, HD, V = 512, 512, 500
G = 3 * HE            # 1536
b = B // N_CORES      # 8 per core
DP = 640              # D padded to 5*128
KX = DP // 128        # 5 x-chunks
KH = HE // 128        # 4 hidden chunks
MG = G // 128         # 12 gate m-tiles (r:0-3, z:4-7, n:8-11)
NRZ = 8               # rz m-tiles
TB = 64               # t-block
NB = T // TB          # 8 blocks
UNROLL = 32
SIG = mybir.ActivationFunctionType.Sigmoid
TANH = mybir.ActivationFunctionType.Tanh
ADD = mybir.AluOpType.add
MUL = mybir.AluOpType.mult
SUB = mybir.AluOpType.subtract

_cache = {}


def _build():
    nc = bacc.Bacc("TRN2", target_bir_lowering=False, debug=False,
                   num_devices=N_CORES)

    def din(name, shape, dt):
        return nc.dram_tensor(name, shape, dt, kind="ExternalInput").ap()

    xT_d = din("xT", [KX, 128, T, b], BF16)
    mask_d = din("mask", [T, 128, KH * b], U8)
    wih_d = din("wih", [KX, 128, G], BF16)
    be_d = din("bias_enc", [MG, 128, 1], F32)
    whh_d = din("whh", [KH, 128, G], BF16)
    bhhn_d = din("bhhn", [4, 128, b], F32)
    w0x_d = din("w0x", [KX, 128, G], BF16)
    w0c_d = din("w0c", [KH, 128, G], BF16)
    b0f_d = din("b0f", [MG, 128, 1], F32)
    b0xn_d = din("b0xn", [4, 128, 1], F32)
    w0mA_d = din("w0mA", [KH, 128, HD], BF16)
    b0m_d = din("b0m", [4, 128, 1], F32)
    w0mh_d = din("w0mh", [KH, 128, HD], BF16)
    w0hm_d = din("w0hm", [KH, 128, G], BF16)
    w1x_d = din("w1x", [KH, 128, G], BF16)
    w1c_d = din("w1c", [KH, 128, G], BF16)
    b1f_d = din("b1f", [MG, 128, 1], F32)
    b1xn_d = din("b1xn", [4, 128, 1], F32)
    w1mA_d = din("w1mA", [KH, 128, HD], BF16)
    b1m_d = din("b1m", [4, 128, 1], F32)
    w1mh_d = din("w1mh", [KH, 128, HD], BF16)
    w1hm_d = din("w1hm", [KH, 128, G], BF16)
    wod_d = din("wod", [KH + 1, 128, 529], BF16)
    woutT_d = din("woutT", [KH, 128, 512], BF16)
    boutT_d = din("boutT", [4, 128, 1], F32)
    wemb_d = din("wemb", [4, 128, 1000], BF16)

    lastT_d = nc.dram_tensor("lastT", [128, KH, b], F32,
                             kind="ExternalOutput").ap()
    outd_d = nc.dram_tensor("outd", [4 * NB, 128, 529], F32,
                            kind="ExternalOutput").ap()
    embd_d = nc.dram_tensor("embd", [E, 4 * NB, 128, V], F32,
                            kind="ExternalOutput").ap()
    xg1_s = nc.dram_tensor("xg1_scratch", [NB, 128, MG, TB, b], BF16).ap()

    def load_w(pool, dram, nk, ncol, tag, dt=BF16):
        t = pool.tile([128, nk, ncol], dt, tag=tag)
        for k in range(nk):
            nc.sync.dma_start(t[:, k, :], dram[k, :, :])
        return t

    def load_b(pool, dram, n, tag):
        t = pool.tile([128, n], F32, tag=tag)
        for m in range(n):
            nc.sync.dma_start(t[:, m:m + 1], dram[m, :, :])
        return t

    with tile.TileContext(nc, trace_sim=False) as tc:
        with tc.tile_pool(name="persist", bufs=1) as pp:
            mask_sb = pp.tile([128, T, KH * b], U8)
            for t in range(T):
                nc.sync.dma_start(mask_sb[:, t, :], mask_d[t, :, :])
            lastT = pp.tile([128, KH, b], F32)
            nc.vector.memset(lastT[:], 0.0)
            ghm0T = pp.tile([128, KH, b], F32)
            ghm1T = pp.tile([128, KH, b], F32)
            cnst0T = pp.tile([128, MG, b], F32)
            cnst1T = pp.tile([128, MG, b], F32)
            lh_bf = pp.tile([128, KH, b], BF16)

            # ---- shared scan-step tail ----
            def gru_tail(wk, tagp, xg, t, ps_rz, ps_n, nbias, hxa, hnxt):
                rzp = wk.tile([128, NRZ * b], F32, tag=tagp + "rzp")
                nc.vector.tensor_tensor(rzp[:], ps_rz[:],
                                        xg[:, 0:NRZ, bass.ds(t, 1), :], ADD)
                rz = wk.tile([128, NRZ * b], F32, tag=tagp + "rzs")
                nc.scalar.activation(rz[:], rzp[:], SIG)
                hnb = wk.tile([128, 4 * b], F32, tag=tagp + "hnb")
                nc.vector.tensor_tensor(hnb[:], ps_n[:], nbias, ADD)
                rhn = wk.tile([128, 4 * b], F32, tag=tagp + "rhn")
                nc.vector.tensor_tensor(rhn[:], hnb[:], rz[:, 0:4 * b], MUL)
                npre = wk.tile([128, 4 * b], F32, tag=tagp + "npre")
                nc.vector.tensor_tensor(npre[:], rhn[:],
                                        xg[:, NRZ:MG, bass.ds(t, 1), :], ADD)
                ng = wk.tile([128, 4 * b], F32, tag=tagp + "ng")
                nc.scalar.activation(ng[:], npre[:], TANH)
                hmn = wk.tile([128, 4 * b], F32, tag=tagp + "hmn")
                nc.vector.tensor_tensor(hmn[:], hxa, ng[:], SUB)
                zh = wk.tile([128, 4 * b], F32, tag=tagp + "zh")
                nc.vector.tensor_tensor(zh[:], rz[:, 4 * b:8 * b], hmn[:], MUL)
                nc.vector.tensor_tensor(
                    hnxt[:].rearrange("p k c -> p (k c)"), zh[:], ng[:], ADD)

            # =================== encoder ===================
            with tc.tile_pool(name="encw", bufs=1) as ewp, \
                 tc.tile_pool(name="encx", bufs=2) as expool, \
                 tc.tile_pool(name="encg", bufs=3) as egp, \
                 tc.tile_pool(name="enchs", bufs=1) as ehp, \
                 tc.tile_pool(name="encwk", bufs=3) as ewk, \
                 tc.tile_pool(name="encpsA", bufs=2, space="PSUM") as epsA, \
                 tc.tile_pool(name="encpsR", bufs=2, space="PSUM") as epsR, \
                 tc.tile_pool(name="encpsN", bufs=2, space="PSUM") as epsN:
                wih_sb = load_w(ewp, wih_d, KX, G, "wih")
                whh_sb = load_w(ewp, whh_d, KH, G, "whh")
                be_sb = load_b(ewp, be_d, MG, "be")
                bhhn_sb = ewp.tile([128, 4, b], F32, tag="bhhn")
                for k4 in range(4):
                    nc.sync.dma_start(bhhn_sb[:, k4, :], bhhn_d[k4, :, :])

                def a_block(blk):
                    xt = expool.tile([128, KX, TB * b], BF16, tag="xt")
                    for k in range(KX):
                        nc.sync.dma_start(
                            xt[:, k, :], xT_d[k, :, blk * TB:(blk + 1) * TB, :])
                    xg = egp.tile([128, MG, TB, b], BF16, tag="xgTB")
                    for m in range(MG):
                        ps = epsA.tile([128, TB * b], F32, tag="big")
                        for k in range(KX):
                            nc.tensor.matmul(
                                ps[:], wih_sb[:, k, m * 128:(m + 1) * 128],
                                xt[:, k, :], start=(k == 0), stop=(k == KX - 1))
                        nc.vector.tensor_scalar_add(
                            xg[:, m, :, :], ps[:], be_sb[:, m:m + 1])
                    return xg

                hA = ehp.tile([128, KH, b], BF16, tag="hA")
                hB = ehp.tile([128, KH, b], BF16, tag="hB")
                nc.vector.memset(hA[:], 0.0)

                def enc_loop(blk, xg):
                    with tc.For_i(0, TB, UNROLL, hint_engines=(mybir.EngineType.PE,)) as iv:
                        for u in range(UNROLL):
                            t = iv + u
                            hcur = hA if u % 2 == 0 else hB
                            hnxt = hB if u % 2 == 0 else hA
                            ps_rz = epsR.tile([128, NRZ * b], F32, tag="rz")
                            ps_n = epsN.tile([128, 4 * b], F32, tag="n")
                            for m in range(MG):
                                dst = (ps_rz[:, m * b:(m + 1) * b] if m < NRZ else
                                       ps_n[:, (m - NRZ) * b:(m - NRZ + 1) * b])
                                for k in range(KH):
                                    nc.tensor.matmul(
                                        dst, whh_sb[:, k, m * 128:(m + 1) * 128],
                                        hcur[:, k, :],
                                        start=(k == 0), stop=(k == KH - 1))
                            gru_tail(ewk, "e", xg, t, ps_rz, ps_n,
                                     bhhn_sb[:], hcur[:], hnxt)
                            nc.vector.copy_predicated(
                                lastT[:].rearrange("p k c -> p (k c)"),
                                mask_sb[:, bass.ds(t + blk * TB, 1), :],
                                hnxt[:].rearrange("p k c -> p (k c)"))

                xg_tiles = [a_block(0), a_block(1)]
                for blk in range(NB):
                    enc_loop(blk, xg_tiles[blk])
                    if blk + 2 < NB:
                        xg_tiles.append(a_block(blk + 2))

                nc.sync.dma_start(lastT_d[:], lastT[:])
                nc.vector.tensor_copy(lh_bf[:], lastT[:])

            # ---- decoder scan step (shared by both layers) ----
            def dec_loop(layer, blk, xg, hAB, pools, wmh_sb, whm_sb,
                         ghm, cnst):
                hpool, wk, psR, psN, psH = pools
                hsb = hpool.tile([128, KH, TB + 1, b], BF16, tag=f"hsD{layer}")
                hA, hB = hAB
                with tc.For_i(0, TB, UNROLL, hint_engines=(mybir.EngineType.PE,)) as iv:
                    for u in range(UNROLL):
                        t = iv + u
                        hcur = hA if u % 2 == 0 else hB
                        hnxt = hB if u % 2 == 0 else hA
                        ps_rz = psR.tile([128, NRZ * b], F32, tag="drz")
                        ps_n = psN.tile([128, 4 * b], F32, tag="dn")
                        ps_hx = psH.tile([128, 4 * b], F32, tag="dhx")
                        for mh in range(KH):
                            for k in range(KH):
                                nc.tensor.matmul(
                                    ps_hx[:, mh * b:(mh + 1) * b],
                                    wmh_sb[:, k, mh * 128:(mh + 1) * 128],
                                    hcur[:, k, :],
                                    start=(k == 0), stop=(k == KH - 1))
                        for m in range(MG):
                            dst = (ps_rz[:, m * b:(m + 1) * b] if m < NRZ else
                                   ps_n[:, (m - NRZ) * b:(m - NRZ + 1) * b])
                            for k in range(KH):
                                nc.tensor.matmul(
                                    dst, whm_sb[:, k, m * 128:(m + 1) * 128],
                                    hcur[:, k, :],
                                    start=(k == 0), stop=(k == KH - 1))
                        hx = wk.tile([128, 4 * b], F32, tag=f"hx{layer}")
                        nc.vector.tensor_tensor(hx[:], ps_hx[:], ghm[:], ADD)
                        gru_tail(wk, f"d{layer}", xg, t, ps_rz, ps_n,
                                 cnst[:, NRZ:MG, :], hx[:], hnxt)
                        nc.scalar.copy(hsb[:, :, bass.ds(t + 1, 1), :],
                                       hnxt[:])
                return hsb

            # =================== decoder layer 0 ===================
            with tc.tile_pool(name="d0w", bufs=1) as dwp, \
                 tc.tile_pool(name="d0x", bufs=2) as dxp, \
                 tc.tile_pool(name="d0g", bufs=3) as dgp, \
                 tc.tile_pool(name="d0hs", bufs=3) as dhp, \
                 tc.tile_pool(name="d0hc", bufs=1) as dhc, \
                 tc.tile_pool(name="d0wk", bufs=3) as dwk, \
                 tc.tile_pool(name="d0e", bufs=2) as dep, \
                 tc.tile_pool(name="d0psA", bufs=2, space="PSUM") as dpsA, \
                 tc.tile_pool(name="d0psR", bufs=2, space="PSUM") as dpsR, \
                 tc.tile_pool(name="d0psN", bufs=2, space="PSUM") as dpsN, \
                 tc.tile_pool(name="d0psH", bufs=2, space="PSUM") as dpsH:
                w0x_sb = load_w(dwp, w0x_d, KX, G, "w0x")
                w0c_sb = load_w(dwp, w0c_d, KH, G, "w0c")
                w0hm_sb = load_w(dwp, w0hm_d, KH, G, "w0hm")
                w0mh_sb = load_w(dwp, w0mh_d, KH, HD, "w0mh")
                w0mA_sb = load_w(dwp, w0mA_d, KH, HD, "w0mA")
                w1x_sb = load_w(dwp, w1x_d, KH, G, "w1x")
                w1c_sb = load_w(dwp, w1c_d, KH, G, "w1c")
                w1mA_sb = load_w(dwp, w1mA_d, KH, HD, "w1mA")
                b0f_sb = load_b(dwp, b0f_d, MG, "b0f")
                b1f_sb = load_b(dwp, b1f_d, MG, "b1f")
                b0xn_sb = load_b(dwp, b0xn_d, 4, "b0xn")
                b1xn_sb = load_b(dwp, b1xn_d, 4, "b1xn")
                b0m_sb = load_b(dwp, b0m_d, 4, "b0m")
                b1m_sb = load_b(dwp, b1m_d, 4, "b1m")

                def consts(wmA, bm, wc, bf_, ghm, cnst):
                    for mh in range(KH):
                        ps = dpsA.tile([128, TB * b], F32, tag="big")
                        for k in range(KH):
                            nc.tensor.matmul(
                                ps[:, 0:b], wmA[:, k, mh * 128:(mh + 1) * 128],
                                lh_bf[:, k, :], start=(k == 0),
                                stop=(k == KH - 1))
                        nc.vector.tensor_scalar_add(
                            ghm[:, mh, :], ps[:, 0:b], bm[:, mh:mh + 1])
                    for m in range(MG):
                        ps = dpsA.tile([128, TB * b], F32, tag="big")
                        for k in range(KH):
                            nc.tensor.matmul(
                                ps[:, 0:b], wc[:, k, m * 128:(m + 1) * 128],
                                lh_bf[:, k, :], start=(k == 0),
                                stop=(k == KH - 1))
                        nc.vector.tensor_scalar_add(
                            cnst[:, m, :], ps[:, 0:b], bf_[:, m:m + 1])

                consts(w0mA_sb, b0m_sb, w0c_sb, b0f_sb, ghm0T, cnst0T)
                consts(w1mA_sb, b1m_sb, w1c_sb, b1f_sb, ghm1T, cnst1T)

                def proj_block(xg, xpart, cnst, bias_n):
                    """xg[m] = xpart-matmuls + (cnst bcast for rz | bias for n)."""
                    xw, xrhs, nkx = xpart
                    for m in range(MG):
                        ps = dpsA.tile([128, TB * b], F32, tag="big")
                        for k in range(nkx):
                            nc.tensor.matmul(
                                ps[:], xw[:, k, m * 128:(m + 1) * 128],
                                xrhs(k), start=(k == 0), stop=(k == nkx - 1))
                        if m < NRZ:
                            nc.vector.tensor_tensor(
                                xg[:, m, :, :], ps[:],
                                cnst[:, m:m + 1, :].broadcast_to([128, TB, b]),
                                ADD)
                        else:
                            nc.vector.tensor_scalar_add(
                                xg[:, m, :, :], ps[:],
                                bias_n[:, m - NRZ:m - NRZ + 1])

                def c_block(blk):
                    xt = dxp.tile([128, KX, TB * b], BF16, tag="xt0")
                    for k in range(KX):
                        nc.sync.dma_start(
                            xt[:, k, :], xT_d[k, :, blk * TB:(blk + 1) * TB, :])
                    xg = dgp.tile([128, MG, TB, b], BF16, tag="x2h0TB")
                    proj_block(xg, (w0x_sb, lambda k: xt[:, k, :], KX),
                               cnst0T, b0xn_sb)
                    return xg

                def e_block(blk, h0sb):
                    xg = dep.tile([128, MG, TB, b], BF16, tag="xg1ev")
                    proj_block(xg,
                               (w1x_sb, lambda k: h0sb[:, k, 1:TB + 1, :], KH),
                               cnst1T, b1xn_sb)
                    nc.sync.dma_start(xg1_s[blk, :, :, :, :], xg[:])

                d0pools = (dhp, dwk, dpsR, dpsN, dpsH)
                h0A = dhc.tile([128, KH, b], BF16, tag="h0A")
                h0B = dhc.tile([128, KH, b], BF16, tag="h0B")
                nc.vector.memset(h0A[:], 0.0)
                xg0_tiles = [c_block(0), c_block(1)]
                for blk in range(NB):
                    h0sb = dec_loop(0, blk, xg0_tiles[blk], (h0A, h0B),
                                    d0pools, w0mh_sb, w0hm_sb, ghm0T, cnst0T)
                    if blk + 2 < NB:
                        xg0_tiles.append(c_block(blk + 2))
                    e_block(blk, h0sb)

            # =================== decoder layer 1 + heads ===================
            with tc.tile_pool(name="d1w", bufs=1) as vwp, \
                 tc.tile_pool(name="d1g", bufs=3) as vgp, \
                 tc.tile_pool(name="d1hs", bufs=3) as vhp, \
                 tc.tile_pool(name="d1hc", bufs=1) as vhc, \
                 tc.tile_pool(name="d1wk", bufs=3) as vwk, \
                 tc.tile_pool(name="d1o", bufs=2) as vop, \
                 tc.tile_pool(name="d1psR", bufs=2, space="PSUM") as vpsR, \
                 tc.tile_pool(name="d1psN", bufs=1, space="PSUM") as vpsN, \
                 tc.tile_pool(name="d1psH", bufs=1, space="PSUM") as vpsH, \
                 tc.tile_pool(name="d1psGA", bufs=2, space="PSUM") as vpsGA, \
                 tc.tile_pool(name="d1psGB", bufs=2, space="PSUM") as vpsGB:
                w1mh_sb = load_w(vwp, w1mh_d, KH, HD, "w1mh")
                w1hm_sb = load_w(vwp, w1hm_d, KH, G, "w1hm")
                wod_sb = load_w(vwp, wod_d, KH + 1, 529, "wod")
                woutT_sb = load_w(vwp, woutT_d, KH, 512, "woutT")
                boutT_sb = load_b(vwp, boutT_d, 4, "boutT")
                wemb_sb = load_w(vwp, wemb_d, 4, 1000, "wemb")
                ones_sb = vwp.tile([128, 128], BF16)
                nc.vector.memset(ones_sb[:], 0.0)
                nc.vector.memset(ones_sb[0:1, :], 1.0)

                def s_block(blk):
                    xg = vgp.tile([128, MG, TB, b], BF16, tag="xg1TB")
                    nc.sync.dma_start(xg[:], xg1_s[blk, :, :, :, :])
                    return xg

                def g_block(blk, hsb):
                    outT = vop.tile([128, 4, TB * b], BF16, tag="outT")
                    for vb in range(4):
                        ps = vpsGA.tile([128, TB * b], F32, tag="gA")
                        for k in range(KH):
                            nc.tensor.matmul(
                                ps[:], woutT_sb[:, k, vb * 128:(vb + 1) * 128],
                                hsb[:, k, 1:TB + 1, :], start=(k == 0),
                                stop=(k == KH - 1))
                        nc.vector.tensor_scalar_add(
                            outT[:, vb, :], ps[:], boutT_sb[:, vb:vb + 1])
                    for j in range(4):
                        rc = blk * 4 + j
                        psA = vpsGA.tile([128, TB * b], F32, tag="gA")
                        psB = vpsGB.tile([128, 17], F32, tag="gB")
                        for k in range(KH):
                            lhsT = hsb[:, k, 1 + 16 * j:1 + 16 * (j + 1), :]
                            nc.tensor.matmul(psA[:, 0:512], lhsT,
                                             wod_sb[:, k, 0:512],
                                             start=(k == 0), stop=False)
                            nc.tensor.matmul(psB[:], lhsT,
                                             wod_sb[:, k, 512:529],
                                             start=(k == 0), stop=False)
                        nc.tensor.matmul(psA[:, 0:512], ones_sb[:],
                                         wod_sb[:, KH, 0:512],
                                         start=False, stop=True)
                        nc.tensor.matmul(psB[:], ones_sb[:],
                                         wod_sb[:, KH, 512:529],
                                         start=False, stop=True)
                        osb = vop.tile([128, 529], F32, tag="osb")
                        nc.scalar.copy(osb[:, 0:512], psA[:, 0:512])
                        nc.vector.tensor_copy(osb[:, 512:529], psB[:])
                        nc.sync.dma_start(outd_d[rc, :, :], osb[:])
                    for p in range(4):
                        for j in range(4):
                            rc = blk * 4 + j
                            psA = vpsGA.tile([128, TB * b], F32, tag="gA")
                            psB = vpsGB.tile([128, V], F32, tag="gB")
                            lhsT = outT[:, p, 128 * j:128 * (j + 1)]
                            nc.tensor.matmul(psA[:, 0:V], lhsT,
                                             wemb_sb[:, p, 0:V],
                                             start=True, stop=True)
                            nc.tensor.matmul(psB[:], lhsT,
                                             wemb_sb[:, p, V:2 * V],
                                             start=True, stop=True)
                            emA = vop.tile([128, V], F32, tag="emA")
                            emB = vop.tile([128, V], F32, tag="emB")
                            nc.scalar.copy(emA[:], psA[:, 0:V])
                            nc.vector.tensor_copy(emB[:], psB[:])
                            nc.sync.dma_start(embd_d[2 * p, rc, :, :], emA[:])
                            nc.sync.dma_start(embd_d[2 * p + 1, rc, :, :], emB[:])

                d1pools = (vhp, vwk, vpsR, vpsN, vpsH)
                h1A = vhc.tile([128, KH, b], BF16, tag="h1A")
                h1B = vhc.tile([128, KH, b], BF16, tag="h1B")
                nc.vector.memset(h1A[:], 0.0)
                xg1_tiles = [s_block(0), s_block(1)]
                for blk in range(NB):
                    h1sb = dec_loop(1, blk, xg1_tiles[blk], (h1A, h1B),
                                    d1pools, w1mh_sb, w1hm_sb, ghm1T, cnst1T)
                    if blk + 2 < NB:
                        xg1_tiles.append(s_block(blk + 2))
                    g_block(blk, h1sb)

    nc.compile()
    return nc


def _prep_inputs(x, Wih, Whh, bih, bhh, W0x, b0x, W0h, b0h, W0m, b0m,
                 W1x, b1x, W1h, b1h, W1m, b1m, Wout, bout, Wdel, bdel,
                 Wemb, bemb, seq_lens):
    f = np.float32
    arrs = dict(x=x, Wih=Wih, Whh=Whh, bih=bih, bhh=bhh, W0x=W0x, b0x=b0x,
                W0h=W0h, b0h=b0h, W0m=W0m, b0m=b0m, W1x=W1x, b1x=b1x,
                W1h=W1h, b1h=b1h, W1m=W1m, b1m=b1m, Wout=Wout, bout=bout,
                Wdel=Wdel, bdel=bdel, Wemb=Wemb)
    a = {k: np.asarray(v, f) for k, v in arrs.items()}
    seq_lens = np.asarray(seq_lens)

    def kview(WT, K):
        return np.ascontiguousarray(WT.reshape(K, 128, -1)).astype(BF)

    def bview(v):
        return np.ascontiguousarray(v.reshape(-1, 128, 1)).astype(f)

    sh = {}
    WihT = np.zeros((DP, G), f); WihT[:D] = a["Wih"].T
    sh["wih"] = kview(WihT, KX)
    be = np.concatenate([(a["bih"] + a["bhh"])[:2 * HE], a["bih"][2 * HE:]])
    sh["bias_enc"] = bview(be)
    sh["whh"] = kview(a["Whh"].T.copy(), KH)
    sh["bhhn"] = np.broadcast_to(a["bhh"][2 * HE:].reshape(4, 128, 1), (4, 128, b)).astype(f).copy()

    for L, (Wx, bx, Wh, bh, Wm, bm) in enumerate([
            (a["W0x"], a["b0x"], a["W0h"], a["b0h"], a["W0m"], a["b0m"]),
            (a["W1x"], a["b1x"], a["W1h"], a["b1h"], a["W1m"], a["b1m"])]):
        WmA, Wmh = Wm[:, :HE], Wm[:, HE:]
        if L == 0:
            WxT = np.zeros((DP, G), f); WxT[:D] = Wx.T
            sh["w0x"] = kview(WxT, KX)
        else:
            sh["w1x"] = kview(Wx.T.copy(), KH)
        Wc = Wh @ WmA
        sh[f"w{L}c"] = kview(Wc.T.copy(), KH)
        bf_ = Wh @ bm + bh
        bf_full = bf_.copy(); bf_full[:2 * HD] += bx[:2 * HD]
        sh[f"b{L}f"] = bview(bf_full)
        sh[f"b{L}xn"] = bview(bx[2 * HD:])
        sh[f"w{L}mA"] = kview(WmA.T.copy(), KH)
        sh[f"b{L}m"] = bview(bm)
        sh[f"w{L}mh"] = kview(Wmh.T.copy(), KH)
        sh[f"w{L}hm"] = kview((Wh @ Wmh).T.copy(), KH)

    wod = np.zeros((KH + 1, 128, 529), f)
    WodT = np.concatenate([a["Wout"].T, a["Wdel"].T], axis=1)
    wod[:KH] = WodT.reshape(KH, 128, 529)
    wod[KH, 0, :] = np.concatenate([a["bout"], a["bdel"]])
    sh["wod"] = wod.astype(BF)
    sh["woutT"] = kview(a["Wout"].T[:, :512].copy(), KH)
    sh["boutT"] = bview(a["bout"][:512])
    wemb = np.zeros((4, 128, 1000), f)
    for p in range(4):
        wemb[p, 0:64, 0:V] = a["Wemb"][2 * p].T
        wemb[p, 64:128, V:2 * V] = a["Wemb"][2 * p + 1].T
    sh["wemb"] = wemb.astype(BF)

    cores = []
    for i in range(N_CORES):
        xs = a["x"][i * b:(i + 1) * b]
        xT = np.zeros((DP, T, b), f)
        xT[:D] = xs.transpose(2, 1, 0)
        lens = seq_lens[i * b:(i + 1) * b].astype(np.int64)
        mask = np.zeros((T, b), np.uint8)
        for j in range(b):
            mask[int(lens[j]) - 1, j] = 1
        cores.append(dict(
            xT=np.ascontiguousarray(xT.reshape(KX, 128, T, b)).astype(BF),
            mask=np.ascontiguousarray(
                np.broadcast_to(mask[:, None, None, :],
                                (T, 128, KH, b)).reshape(T, 128, KH * b)
            ).astype(np.uint8),
            **sh))
    return cores


def kernel(**inputs):
    if "nc" not in _cache:
        _cache["nc"] = _build()
    nc = _cache["nc"]
    cores = _prep_inputs(**inputs)
    res = run_bass_kernel_spmd(nc, cores, core_ids=list(range(N_CORES)))

    out = np.zeros((B, T - 1, D), np.float32)
    delta = np.zeros((B, T - 1), np.float32)
    emb = np.zeros((E, B, T - 1, V), np.float32)
    lh = np.zeros((B, HE), np.float32)
    bemb = np.asarray(inputs["bemb"], np.float32)
    for i in range(N_CORES):
        r = res.results[i]
        od = r["outd"].reshape(T, b, 529)
        out[i * b:(i + 1) * b] = od[:T - 1, :, :D].transpose(1, 0, 2)
        delta[i * b:(i + 1) * b] = od[:T - 1, :, 528].transpose(1, 0)
        ed = r["embd"].reshape(E, T, b, V)
        emb[:, i * b:(i + 1) * b] = ed[:, :T - 1].transpose(0, 2, 1, 3)
        lt = r["lastT"]
        lh[i * b:(i + 1) * b] = lt.transpose(2, 1, 0).reshape(b, HE)
    emb += bemb[:, None, None, :]
    return out, delta, emb, lh


# revision 17
# speedup vs baseline: 1.0478x; 1.0478x over previous
"""Trainium2 Bass kernel for nn_GRUGen: GRU encoder + 2-layer decoder GRU with
global mixing + output heads. 8-way batch-sharded SPMD (b=8 per core).

Layout convention: everything transposed — feature dim on partitions, batch on
free dim. Recurrent matmuls are weight-stationary (bf16, FWL) producing gates
in [gate-chunk partitions, batch] layout so elementwise runs at full partition
utilization. Decoder runs as two full scans (layer 0, then layer 1) with the
inter-layer W1x projection done as blocked parallel matmuls; the per-sample
global-mixing constants are folded on the host (W0h@W0m etc).
"""
import sys
sys.path.insert(0, "/opt/trn_rl_repo")

import numpy as np
import ml_dtypes

import concourse.bass as bass
import concourse.mybir as mybir
import concourse.tile as tile
from concourse import bacc
from concourse.bass_utils import run_bass_kernel_spmd

F32 = mybir.dt.float32
BF16 = mybir.dt.bfloat16
U8 = mybir.dt.uint8
BF = ml_dtypes.bfloat16

N_CORES = 8
B, T, D = 64, 512, 528
E, Fdim = 8, 64
HE, HD, V = 512, 512, 500
G = 3 * HE            # 1536
b = B // N_CORES      # 8 per core
DP = 640              # D padded to 5*128
KX = DP // 128        # 5 x-chunks
KH = HE // 128        # 4 hidden chunks
MG = G // 128         # 12 gate m-tiles (r:0-3, z:4-7, n:8-11)
NRZ = 8               # rz m-tiles
TB = 64               # t-block
NB = T // TB          # 8 blocks
UNROLL = 32
SIG = mybir.ActivationFunctionType.Sigmoid
TANH = mybir.ActivationFunctionType.Tanh
ADD = mybir.AluOpType.add
MUL = mybir.AluOpType.mult
SUB = mybir.AluOpType.subtract

_cache = {}


def _build():
    nc = bacc.Bacc("TRN2", target_bir_lowering=False, debug=False,
                   num_devices=N_CORES)

    def din(name, shape, dt):
        return nc.dram_tensor(name, shape, dt, kind="ExternalInput").ap()

    xT_d = din("xT", [KX, 128, T, b], BF16)
    mask_d = din("mask", [T, 128, KH * b], U8)
    wih_d = din("wih", [KX, 128, G], BF16)
    be_d = din("bias_enc", [MG, 128, 1], F32)
    whh_d = din("whh", [KH, 128, G], BF16)
    bhhn_d = din("bhhn", [4, 128, b], F32)
    w0x_d = din("w0x", [KX, 128, G], BF16)
    w0c_d = din("w0c", [KH, 128, G], BF16)
    b0f_d = din("b0f", [MG, 128, 1], F32)
    b0xn_d = din("b0xn", [4, 128, 1], F32)
    w0mA_d = din("w0mA", [KH, 128, HD], BF16)
    b0m_d = din("b0m", [4, 128, 1], F32)
    w0mh_d = din("w0mh", [KH, 128, HD], BF16)
    w0hm_d = din("w0hm", [KH, 128, G], BF16)
    w1x_d = din("w1x", [KH, 128, G], BF16)
    w1c_d = din("w1c", [KH, 128, G], BF16)
    b1f_d = din("b1f", [MG, 128, 1], F32)
    b1xn_d = din("b1xn", [4, 128, 1], F32)
    w1mA_d = din("w1mA", [KH, 128, HD], BF16)
    b1m_d = din("b1m", [4, 128, 1], F32)
    w1mh_d = din("w1mh", [KH, 128, HD], BF16)
    w1hm_d = din("w1hm", [KH, 128, G], BF16)
    wod_d = din("wod", [KH + 1, 128, 529], BF16)
    woutT_d = din("woutT", [KH, 128, 512], BF16)
    boutT_d = din("boutT", [4, 128, 1], F32)
    wemb_d = din("wemb", [4, 128, 1000], BF16)

    lastT_d = nc.dram_tensor("lastT", [128, KH, b], F32,
                             kind="ExternalOutput").ap()
    outd_d = nc.dram_tensor("outd", [4 * NB, 128, 529], F32,
                            kind="ExternalOutput").ap()
    embd_d = nc.dram_tensor("embd", [E, 4 * NB, 128, V], F32,
                            kind="ExternalOutput").ap()
    xg1_s = nc.dram_tensor("xg1_scratch", [NB, 128, MG, TB, b], BF16).ap()

    def load_w(pool, dram, nk, ncol, tag, dt=BF16):
        t = pool.tile([128, nk, ncol], dt, tag=tag)
        for k in range(nk):
            nc.sync.dma_start(t[:, k, :], dram[k, :, :])
        return t

    def load_b(pool, dram, n, tag):
        t = pool.tile([128, n], F32, tag=tag)
        for m in range(n):
            nc.sync.dma_start(t[:, m:m + 1], dram[m, :, :])
        return t

    with tile.TileContext(nc, trace_sim=False) as tc:
        with tc.tile_pool(name="persist", bufs=1) as pp:
            mask_sb = pp.tile([128, T, KH * b], U8)
            for t in range(T):
                nc.sync.dma_start(mask_sb[:, t, :], mask_d[t, :, :])
            lastT = pp.tile([128, KH, b], F32)
            nc.vector.memset(lastT[:], 0.0)
            ghm0T = pp.tile([128, KH, b], F32)
            ghm1T = pp.tile([128, KH, b], F32)
            cnst0T = pp.tile([128, MG, b], F32)
            cnst1T = pp.tile([128, MG, b], F32)
            lh_bf = pp.tile([128, KH, b], BF16)

            # ---- shared scan-step tail ----
            def gru_tail(wk, tagp, xg, t, ps_rz, ps_n, nbias, hxa, hnxt):
                rzp = wk.tile([128, NRZ * b], F32, tag=tagp + "rzp")
                nc.vector.tensor_tensor(rzp[:], ps_rz[:],
                                        xg[:, 0:NRZ, bass.ds(t, 1), :], ADD)
                rz = wk.tile([128, NRZ * b], F32, tag=tagp + "rzs")
                nc.scalar.activation(rz[:], rzp[:], SIG)
                hnb = wk.tile([128, 4 * b], F32, tag=tagp + "hnb")
                nc.vector.tensor_tensor(hnb[:], ps_n[:], nbias, ADD)
                rhn = wk.tile([128, 4 * b], F32, tag=tagp + "rhn")
                nc.vector.tensor_tensor(rhn[:], hnb[:], rz[:, 0:4 * b], MUL)
                npre = wk.tile([128, 4 * b], F32, tag=tagp + "npre")
                nc.vector.tensor_tensor(npre[:], rhn[:],
                                        xg[:, NRZ:MG, bass.ds(t, 1), :], ADD)
                ng = wk.tile([128, 4 * b], F32, tag=tagp + "ng")
                nc.scalar.activation(ng[:], npre[:], TANH)
                hmn = wk.tile([128, 4 * b], F32, tag=tagp + "hmn")
                nc.vector.tensor_tensor(hmn[:], hxa, ng[:], SUB)
                zh = wk.tile([128, 4 * b], F32, tag=tagp + "zh")
                nc.vector.tensor_tensor(zh[:], rz[:, 4 * b:8 * b], hmn[:], MUL)
                nc.vector.tensor_tensor(
                    hnxt[:].rearrange("p k c -> p (k c)"), zh[:], ng[:], ADD)

            # =================== encoder ===================
            with tc.tile_pool(name="encw", bufs=1) as ewp, \
                 tc.tile_pool(name="encx", bufs=2) as expool, \
                 tc.tile_pool(name="encg", bufs=3) as egp, \
                 tc.tile_pool(name="enchs", bufs=1) as ehp, \
                 tc.tile_pool(name="encwk", bufs=3) as ewk, \
                 tc.tile_pool(name="encpsA", bufs=2, space="PSUM") as epsA, \
                 tc.tile_pool(name="encpsR", bufs=2, space="PSUM") as epsR, \
                 tc.tile_pool(name="encpsN", bufs=2, space="PSUM") as epsN:
                wih_sb = load_w(ewp, wih_d, KX, G, "wih")
                whh_sb = load_w(ewp, whh_d, KH, G, "whh")
                be_sb = load_b(ewp, be_d, MG, "be")
                bhhn_sb = ewp.tile([128, 4, b], F32, tag="bhhn")
                for k4 in range(4):
                    nc.sync.dma_start(bhhn_sb[:, k4, :], bhhn_d[k4, :, :])

                def a_block(blk):
                    xt = expool.tile([128, KX, TB * b], BF16, tag="xt")
                    for k in range(KX):
                        nc.sync.dma_start(
                            xt[:, k, :], xT_d[k, :, blk * TB:(blk + 1) * TB, :])
                    xg = egp.tile([128, MG, TB, b], BF16, tag="xgTB")
                    for m in range(MG):
                        ps = epsA.tile([128, TB * b], F32, tag="big")
                        for k in range(KX):
                            nc.tensor.matmul(
                                ps[:], wih_sb[:, k, m * 128:(m + 1) * 128],
                                xt[:, k, :], start=(k == 0), stop=(k == KX - 1))
                        nc.vector.tensor_scalar_add(
                            xg[:, m, :, :], ps[:], be_sb[:, m:m + 1])
                    return xg

                hA = ehp.tile([128, KH, b], BF16, tag="hA")
                hB = ehp.tile([128, KH, b], BF16, tag="hB")
                nc.vector.memset(hA[:], 0.0)

                def enc_loop(blk, xg):
                    with tc.For_i(0, TB, UNROLL, hint_engines=(mybir.EngineType.PE,)) as iv:
                        for u in range(UNROLL):
                            t = iv + u
                            hcur = hA if u % 2 == 0 else hB
                            hnxt = hB if u % 2 == 0 else hA
                            ps_rz = epsR.tile([128, NRZ * b], F32, tag="rz")
                            ps_n = epsN.tile([128, 4 * b], F32, tag="n")
                            for m in range(MG):
                                dst = (ps_rz[:, m * b:(m + 1) * b] if m < NRZ else
                                       ps_n[:, (m - NRZ) * b:(m - NRZ + 1) * b])
                                for k in range(KH):
                                    nc.tensor.matmul(
                                        dst, whh_sb[:, k, m * 128:(m + 1) * 128],
                                        hcur[:, k, :],
                                        start=(k == 0), stop=(k == KH - 1))
                            gru_tail(ewk, "e", xg, t, ps_rz, ps_n,
                                     bhhn_sb[:], hcur[:], hnxt)
                            nc.vector.copy_predicated(
                                lastT[:].rearrange("p k c -> p (k c)"),
                                mask_sb[:, bass.ds(t + blk * TB, 1), :],
                                hnxt[:].rearrange("p k c -> p (k c)"))

                xg_tiles = [a_block(0), a_block(1)]
                for blk in range(NB):
                    enc_loop(blk, xg_tiles[blk])
                    if blk + 2 < NB:
                        xg_tiles.append(a_block(blk + 2))

                nc.sync.dma_start(lastT_d[:], lastT[:])
                nc.vector.tensor_copy(lh_bf[:], lastT[:])

            # ---- decoder scan step (shared by both layers) ----
            def dec_loop(layer, blk, xg, hAB, pools, wmh_sb, whm_sb,
                         ghm, cnst):
                hpool, wk, psR, psN, psH = pools
                hsb = hpool.tile([128, KH, TB + 1, b], BF16, tag=f"hsD{layer}")
                hA, hB = hAB
                with tc.For_i(0, TB, UNROLL, hint_engines=(mybir.EngineType.PE,)) as iv:
                    for u in range(UNROLL):
                        t = iv + u
                        hcur = hA if u % 2 == 0 else hB
                        hnxt = hB if u % 2 == 0 else hA
                        ps_rz = psR.tile([128, NRZ * b], F32, tag="drz")
                        ps_n = psN.tile([128, 4 * b], F32, tag="dn")
                        ps_hx = psH.tile([128, 4 * b], F32, tag="dhx")
                        for m in range(MG):
                            dst = (ps_rz[:, m * b:(m + 1) * b] if m < NRZ else
                                   ps_n[:, (m - NRZ) * b:(m - NRZ + 1) * b])
                            for k in range(KH):
                                nc.tensor.matmul(
                                    dst, whm_sb[:, k, m * 128:(m + 1) * 128],
                                    hcur[:, k, :],
                                    start=(k == 0), stop=(k == KH - 1))
                        for mh in range(KH):
                            for k in range(KH):
                                nc.tensor.matmul(
                                    ps_hx[:, mh * b:(mh + 1) * b],
                                    wmh_sb[:, k, mh * 128:(mh + 1) * 128],
                                    hcur[:, k, :],
                                    start=(k == 0), stop=(k == KH - 1))
                        hx = wk.tile([128, 4 * b], F32, tag=f"hx{layer}")
                        nc.vector.tensor_tensor(hx[:], ps_hx[:], ghm[:], ADD)
                        gru_tail(wk, f"d{layer}", xg, t, ps_rz, ps_n,
                                 cnst[:, NRZ:MG, :], hx[:], hnxt)
                        nc.scalar.copy(hsb[:, :, bass.ds(t + 1, 1), :],
                                       hnxt[:])
                return hsb

            # =================== decoder layer 0 ===================
            with tc.tile_pool(name="d0w", bufs=1) as dwp, \
                 tc.tile_pool(name="d0x", bufs=2) as dxp, \
                 tc.tile_pool(name="d0g", bufs=3) as dgp, \
                 tc.tile_pool(name="d0hs", bufs=3) as dhp, \
                 tc.tile_pool(name="d0hc", bufs=1) as dhc, \
                 tc.tile_pool(name="d0wk", bufs=3) as dwk, \
                 tc.tile_pool(name="d0e", bufs=2) as dep, \
                 tc.tile_pool(name="d0psA", bufs=2, space="PSUM") as dpsA, \
                 tc.tile_pool(name="d0psR", bufs=2, space="PSUM") as dpsR, \
                 tc.tile_pool(name="d0psN", bufs=2, space="PSUM") as dpsN, \
                 tc.tile_pool(name="d0psH", bufs=2, space="PSUM") as dpsH:
                w0x_sb = load_w(dwp, w0x_d, KX, G, "w0x")
                w0c_sb = load_w(dwp, w0c_d, KH, G, "w0c")
                w0hm_sb = load_w(dwp, w0hm_d, KH, G, "w0hm")
                w0mh_sb = load_w(dwp, w0mh_d, KH, HD, "w0mh")
                w0mA_sb = load_w(dwp, w0mA_d, KH, HD, "w0mA")
                w1x_sb = load_w(dwp, w1x_d, KH, G, "w1x")
                w1c_sb = load_w(dwp, w1c_d, KH, G, "w1c")
                w1mA_sb = load_w(dwp, w1mA_d, KH, HD, "w1mA")
                b0f_sb = load_b(dwp, b0f_d, MG, "b0f")
                b1f_sb = load_b(dwp, b1f_d, MG, "b1f")
                b0xn_sb = load_b(dwp, b0xn_d, 4, "b0xn")
                b1xn_sb = load_b(dwp, b1xn_d, 4, "b1xn")
                b0m_sb = load_b(dwp, b0m_d, 4, "b0m")
                b1m_sb = load_b(dwp, b1m_d, 4, "b1m")

                def consts(wmA, bm, wc, bf_, ghm, cnst):
                    for mh in range(KH):
                        ps = dpsA.tile([128, TB * b], F32, tag="big")
                        for k in range(KH):
                            nc.tensor.matmul(
                                ps[:, 0:b], wmA[:, k, mh * 128:(mh + 1) * 128],
                                lh_bf[:, k, :], start=(k == 0),
                                stop=(k == KH - 1))
                        nc.vector.tensor_scalar_add(
                            ghm[:, mh, :], ps[:, 0:b], bm[:, mh:mh + 1])
                    for m in range(MG):
                        ps = dpsA.tile([128, TB * b], F32, tag="big")
                        for k in range(KH):
                            nc.tensor.matmul(
                                ps[:, 0:b], wc[:, k, m * 128:(m + 1) * 128],
                                lh_bf[:, k, :], start=(k == 0),
                                stop=(k == KH - 1))
                        nc.vector.tensor_scalar_add(
                            cnst[:, m, :], ps[:, 0:b], bf_[:, m:m + 1])

                consts(w0mA_sb, b0m_sb, w0c_sb, b0f_sb, ghm0T, cnst0T)
                consts(w1mA_sb, b1m_sb, w1c_sb, b1f_sb, ghm1T, cnst1T)

                def proj_block(xg, xpart, cnst, bias_n):
                    """xg[m] = xpart-matmuls + (cnst bcast for rz | bias for n)."""
                    xw, xrhs, nkx = xpart
                    for m in range(MG):
                        ps = dpsA.tile([128, TB * b], F32, tag="big")
                        for k in range(nkx):
                            nc.tensor.matmul(
                                ps[:], xw[:, k, m * 128:(m + 1) * 128],
                                xrhs(k), start=(k == 0), stop=(k == nkx - 1))
                        if m < NRZ:
                            nc.vector.tensor_tensor(
                                xg[:, m, :, :], ps[:],
                                cnst[:, m:m + 1, :].broadcast_to([128, TB, b]),
                                ADD)
                        else:
                            nc.vector.tensor_scalar_add(
                                xg[:, m, :, :], ps[:],
                                bias_n[:, m - NRZ:m - NRZ + 1])

                def c_block(blk):
                    xt = dxp.tile([128, KX, TB * b], BF16, tag="xt0")
                    for k in range(KX):
                        nc.sync.dma_start(
                            xt[:, k, :], xT_d[k, :, blk * TB:(blk + 1) * TB, :])
                    xg = dgp.tile([128, MG, TB, b], BF16, tag="x2h0TB")
                    proj_block(xg, (w0x_sb, lambda k: xt[:, k, :], KX),
                               cnst0T, b0xn_sb)
                    return xg

                def e_block(blk, h0sb):
                    xg = dep.tile([128, MG, TB, b], BF16, tag="xg1ev")
                    proj_block(xg,
                               (w1x_sb, lambda k: h0sb[:, k, 1:TB + 1, :], KH),
                               cnst1T, b1xn_sb)
                    nc.sync.dma_start(xg1_s[blk, :, :, :, :], xg[:])

                d0pools = (dhp, dwk, dpsR, dpsN, dpsH)
                h0A = dhc.tile([128, KH, b], BF16, tag="h0A")
                h0B = dhc.tile([128, KH, b], BF16, tag="h0B")
                nc.vector.memset(h0A[:], 0.0)
                xg0_tiles = [c_block(0), c_block(1)]
                for blk in range(NB):
                    h0sb = dec_loop(0, blk, xg0_tiles[blk], (h0A, h0B),
                                    d0pools, w0mh_sb, w0hm_sb, ghm0T, cnst0T)
                    if blk + 2 < NB:
                        xg0_tiles.append(c_block(blk + 2))
                    e_block(blk, h0sb)

            # =================== decoder layer 1 + heads ===================
            with tc.tile_pool(name="d1w", bufs=1) as vwp, \
                 tc.tile_pool(name="d1g", bufs=3) as vgp, \
                 tc.tile_pool(name="d1hs", bufs=3) as vhp, \
                 tc.tile_pool(name="d1hc", bufs=1) as vhc, \
                 tc.tile_pool(name="d1wk", bufs=3) as vwk, \
                 tc.tile_pool(name="d1o", bufs=2) as vop, \
                 tc.tile_pool(name="d1psR", bufs=2, space="PSUM") as vpsR, \
                 tc.tile_pool(name="d1psN", bufs=1, space="PSUM") as vpsN, \
                 tc.tile_pool(name="d1psH", bufs=1, space="PSUM") as vpsH, \
                 tc.tile_pool(name="d1psGA", bufs=2, space="PSUM") as vpsGA, \
                 tc.tile_pool(name="d1psGB", bufs=2, space="PSUM") as vpsGB:
                w1mh_sb = load_w(vwp, w1mh_d, KH, HD, "w1mh")
                w1hm_sb = load_w(vwp, w1hm_d, KH, G, "w1hm")
                wod_sb = load_w(vwp, wod_d, KH + 1, 529, "wod")
                woutT_sb = load_w(vwp, woutT_d, KH, 512, "woutT")
                boutT_sb = load_b(vwp, boutT_d, 4, "boutT")
                wemb_sb = load_w(vwp, wemb_d, 4, 1000, "wemb")
                ones_sb = vwp.tile([128, 128], BF16)
                nc.vector.memset(ones_sb[:], 0.0)
                nc.vector.memset(ones_sb[0:1, :], 1.0)

                def s_block(blk):
                    xg = vgp.tile([128, MG, TB, b], BF16, tag="xg1TB")
                    nc.sync.dma_start(xg[:], xg1_s[blk, :, :, :, :])
                    return xg

                def g_block(blk, hsb):
                    outT = vop.tile([128, 4, TB * b], BF16, tag="outT")
                    for vb in range(4):
                        ps = vpsGA.tile([128, TB * b], F32, tag="gA")
                        for k in range(KH):
                            nc.tensor.matmul(
                                ps[:], woutT_sb[:, k, vb * 128:(vb + 1) * 128],
                                hsb[:, k, 1:TB + 1, :], start=(k == 0),
                                stop=(k == KH - 1))
                        nc.vector.tensor_scalar_add(
                            outT[:, vb, :], ps[:], boutT_sb[:, vb:vb + 1])
                    for j in range(4):
                        rc = blk * 4 + j
                        psA = vpsGA.tile([128, TB * b], F32, tag="gA")
                        psB = vpsGB.tile([128, 17], F32, tag="gB")
                        for k in range(KH):
                            lhsT = hsb[:, k, 1 + 16 * j:1 + 16 * (j + 1), :]
                            nc.tensor.matmul(psA[:, 0:512], lhsT,
                                             wod_sb[:, k, 0:512],
                                             start=(k == 0), stop=False)
                            nc.tensor.matmul(psB[:], lhsT,
                                             wod_sb[:, k, 512:529],
                                             start=(k == 0), stop=False)
                        nc.tensor.matmul(psA[:, 0:512], ones_sb[:],
                                         wod_sb[:, KH, 0:512],
                                         start=False, stop=True)
                        nc.tensor.matmul(psB[:], ones_sb[:],
                                         wod_sb[:, KH, 512:529],
                                         start=False, stop=True)
                        osb = vop.tile([128, 529], F32, tag="osb")
                        nc.scalar.copy(osb[:, 0:512], psA[:, 0:512])
                        nc.vector.tensor_copy(osb[:, 512:529], psB[:])
                        nc.sync.dma_start(outd_d[rc, :, :], osb[:])
                    for p in range(4):
                        for j in range(4):
                            rc = blk * 4 + j
                            psA = vpsGA.tile([128, TB * b], F32, tag="gA")
                            psB = vpsGB.tile([128, V], F32, tag="gB")
                            lhsT = outT[:, p, 128 * j:128 * (j + 1)]
                            nc.tensor.matmul(psA[:, 0:V], lhsT,
                                             wemb_sb[:, p, 0:V],
                                             start=True, stop=True)
                            nc.tensor.matmul(psB[:], lhsT,
                                             wemb_sb[:, p, V:2 * V],
                                             start=True, stop=True)
                            emA = vop.tile([128, V], F32, tag="emA")
                            emB = vop.tile([128, V], F32, tag="emB")
                            nc.scalar.copy(emA[:], psA[:, 0:V])
                            nc.vector.tensor_copy(emB[:], psB[:])
                            nc.sync.dma_start(embd_d[2 * p, rc, :, :], emA[:])
                            nc.sync.dma_start(embd_d[2 * p + 1, rc, :, :], emB[:])

                d1pools = (vhp, vwk, vpsR, vpsN, vpsH)
                h1A = vhc.tile([128, KH, b], BF16, tag="h1A")
                h1B = vhc.tile([128, KH, b], BF16, tag="h1B")
                nc.vector.memset(h1A[:], 0.0)
                xg1_tiles = [s_block(0), s_block(1)]
                for blk in range(NB):
                    h1sb = dec_loop(1, blk, xg1_tiles[blk], (h1A, h1B),
                                    d1pools, w1mh_sb, w1hm_sb, ghm1T, cnst1T)
                    if blk + 2 < NB:
                        xg1_tiles.append(s_block(blk + 2))
                    g_block(blk, h1sb)

    nc.compile()
    return nc


def _prep_inputs(x, Wih, Whh, bih, bhh, W0x, b0x, W0h, b0h, W0m, b0m,
                 W1x, b1x, W1h, b1h, W1m, b1m, Wout, bout, Wdel, bdel,
                 Wemb, bemb, seq_lens):
    f = np.float32
    arrs = dict(x=x, Wih=Wih, Whh=Whh, bih=bih, bhh=bhh, W0x=W0x, b0x=b0x,
                W0h=W0h, b0h=b0h, W0m=W0m, b0m=b0m, W1x=W1x, b1x=b1x,
                W1h=W1h, b1h=b1h, W1m=W1m, b1m=b1m, Wout=Wout, bout=bout,
                Wdel=Wdel, bdel=bdel, Wemb=Wemb)
    a = {k: np.asarray(v, f) for k, v in arrs.items()}
    seq_lens = np.asarray(seq_lens)

    def kview(WT, K):
        return np.ascontiguousarray(WT.reshape(K, 128, -1)).astype(BF)

    def bview(v):
        return np.ascontiguousarray(v.reshape(-1, 128, 1)).astype(f)

    sh = {}
    WihT = np.zeros((DP, G), f); WihT[:D] = a["Wih"].T
    sh["wih"] = kview(WihT, KX)
    be = np.concatenate([(a["bih"] + a["bhh"])[:2 * HE], a["bih"][2 * HE:]])
    sh["bias_enc"] = bview(be)
    sh["whh"] = kview(a["Whh"].T.copy(), KH)
    sh["bhhn"] = np.broadcast_to(a["bhh"][2 * HE:].reshape(4, 128, 1), (4, 128, b)).astype(f).copy()

    for L, (Wx, bx, Wh, bh, Wm, bm) in enumerate([
            (a["W0x"], a["b0x"], a["W0h"], a["b0h"], a["W0m"], a["b0m"]),
            (a["W1x"], a["b1x"], a["W1h"], a["b1h"], a["W1m"], a["b1m"])]):
        WmA, Wmh = Wm[:, :HE], Wm[:, HE:]
        if L == 0:
            WxT = np.zeros((DP, G), f); WxT[:D] = Wx.T
            sh["w0x"] = kview(WxT, KX)
        else:
            sh["w1x"] = kview(Wx.T.copy(), KH)
        Wc = Wh @ WmA
        sh[f"w{L}c"] = kview(Wc.T.copy(), KH)
        bf_ = Wh @ bm + bh
        bf_full = bf_.copy(); bf_full[:2 * HD] += bx[:2 * HD]
        sh[f"b{L}f"] = bview(bf_full)
        sh[f"b{L}xn"] = bview(bx[2 * HD:])
        sh[f"w{L}mA"] = kview(WmA.T.copy(), KH)
        sh[f"b{L}m"] = bview(bm)
        sh[f"w{L}mh"] = kview(Wmh.T.copy(), KH)
        sh[f"w{L}hm"] = kview((Wh @ Wmh).T.copy(), KH)

    wod = np.zeros((KH + 1, 128, 529), f)
    WodT = np.concatenate([a["Wout"].T, a["Wdel"].T], axis=1)
    wod[:KH] = WodT.reshape(KH, 128, 529)
    wod[KH, 0, :] = np.concatenate([a["bout"], a["bdel"]])
    sh["wod"] = wod.astype(BF)
    sh["woutT"] = kview(a["Wout"].T[:, :512].copy(), KH)
    sh["boutT"] = bview(a["bout"][:512])
    wemb = np.zeros((4, 128, 1000), f)
    for p in range(4):
        wemb[p, 0:64, 0:V] = a["Wemb"][2 * p].T
        wemb[p, 64:128, V:2 * V] = a["Wemb"][2 * p + 1].T
    sh["wemb"] = wemb.astype(BF)

    cores = []
    for i in range(N_CORES):
        xs = a["x"][i * b:(i + 1) * b]
        xT = np.zeros((DP, T, b), f)
        xT[:D] = xs.transpose(2, 1, 0)
        lens = seq_lens[i * b:(i + 1) * b].astype(np.int64)
        mask = np.zeros((T, b), np.uint8)
        for j in range(b):
            mask[int(lens[j]) - 1, j] = 1
        cores.append(dict(
            xT=np.ascontiguousarray(xT.reshape(KX, 128, T, b)).astype(BF),
            mask=np.ascontiguousarray(
                np.broadcast_to(mask[:, None, None, :],
                                (T, 128, KH, b)).reshape(T, 128, KH * b)
            ).astype(np.uint8),
            **sh))
    return cores


def kernel(**inputs):
    if "nc" not in _cache:
        _cache["nc"] = _build()
    nc = _cache["nc"]
    cores = _prep_inputs(**inputs)
    res = run_bass_kernel_spmd(nc, cores, core_ids=list(range(N_CORES)))

    out = np.zeros((B, T - 1, D), np.float32)
    delta = np.zeros((B, T - 1), np.float32)
    emb = np.zeros((E, B, T - 1, V), np.float32)
    lh = np.zeros((B, HE), np.float32)
    bemb = np.asarray(inputs["bemb"], np.float32)
    for i in range(N_CORES):
        r = res.results[i]
        od = r["outd"].reshape(T, b, 529)
        out[i * b:(i + 1) * b] = od[:T - 1, :, :D].transpose(1, 0, 2)
        delta[i * b:(i + 1) * b] = od[:T - 1, :, 528].transpose(1, 0)
        ed = r["embd"].reshape(E, T, b, V)
        emb[:, i * b:(i + 1) * b] = ed[:, :T - 1].transpose(0, 2, 1, 3)
        lt = r["lastT"]
        lh[i * b:(i + 1) * b] = lt.transpose(2, 1, 0).reshape(b, HE)
    emb += bemb[:, None, None, :]
    return out, delta, emb, lh
